# revision 35
# baseline (speedup 1.0000x reference)
"""FALCON exists-restriction loss kernel for trn2 (Bass/Tile).

Math: the reference computes
    loss = mean_b -log(1 - max_i max_j sig(s1[j]+rw1[b]+fcb+s2[i]) * sig(s2[j]+cw1[b]+fcb) + eps)
with s1 = e_all@w1, s2 = e_all@w2.  Since sigmoid is strictly increasing and
the c_fs factor is positive, max_i commutes inward:  max_i only affects the
r_fs term through s2[i], so max_i max_j f(i,j) = max_j f(argmax_i s2, j).
The O(B*N^2) grid collapses to O(B*N) exactly.

Device layout: j on partitions (chunks of 128), per-chunk matmul with
stationary e-chunk [128d,128j] (bf16) and moving WB [128d,17] (w1 x8 cols,
w2 x8 cols, w2 raw col).  A k=1 matmul accumulates per-(c,b) biases into the
grid.  exp-form product (1+e^-u)(1+e^-v), min-reduced, then
loss_b = ln(mn) - ln(mn-1) (Ln and Exp share one ACT table -> no table switch).

Sharding: replicated across the 8 cores (an 8-core AllReduce floor ~10us
exceeds the whole kernel; each core reads its own 1MB copy instead).
"""

import os
import sys

import numpy as np

for _p in ("/opt/trn_rl_repo",):
    if _p not in sys.path:
        sys.path.insert(0, _p)

# The device kernel runs through PJRT on the axon backend; a JAX_PLATFORMS=cpu
# pin (common in bench wrappers for the *reference* side) would hide the
# NeuronCores from run_bass_kernel_spmd.  Drop such a pin before jax inits.
if "jax" not in sys.modules and os.environ.get("JAX_PLATFORMS", "").strip() == "cpu":
    del os.environ["JAX_PLATFORMS"]

import ml_dtypes

D = 128
B = 8
N_ENT = 4096
ANON = 4
N = N_ENT + ANON          # 4100
GC = 17                   # grid cols per chunk: 8 u, 8 v, 1 raw s2
NCHUNKS = 33              # 32 full 128-wide chunks + one 4-wide
NQ = 11                   # chunks per psum tile
NTILES = 3
TCOLS = NQ * GC           # 187
ETILE_J = NQ * 128        # 1408 j per e tile
CCOLS = 289               # consts width (280 f32 + 9 f32 holding 18 bf16 wb cols)

EDMA_SPLIT = 1
LAST_EXEC_NS = None
_CACHE = {}
DEBUG_OUTPUTS = False


def _build_device_inputs(x, anon_e_emb, c_table, r_table, e_table, fc_w, fc_b):
    e_all = np.concatenate(
        [np.asarray(e_table, np.float32), np.asarray(anon_e_emb, np.float32)], axis=0
    )  # [N, D]
    fc_w = np.asarray(fc_w, np.float32)
    w1 = fc_w[0, :D]
    w2 = fc_w[0, D:]
    xi = np.asarray(x).astype(np.int64)
    r_emb = np.asarray(r_table, np.float32)[xi[:, 0]]  # [B, D]
    c_emb = np.asarray(c_table, np.float32)[xi[:, 1]]  # [B, D]
    fcb = np.float32(np.asarray(fc_b, np.float32).reshape(-1)[0])

    e_t = np.ascontiguousarray(e_all.T).astype(ml_dtypes.bfloat16)  # [D, N]

    wb = np.zeros((D, 18), np.float32)
    wb[:, 0:8] = w1[:, None]
    wb[:, 8:16] = w2[:, None]
    wb[:, 16] = w2
    wb = np.ascontiguousarray(wb.astype(ml_dtypes.bfloat16))  # [D, 18] bf16

    consts = np.zeros((D, CCOLS), np.float32)
    consts[:, 0] = w1                                   # w1 column (f32)
    consts[:, 1:9] = r_emb.T                            # rT
    consts[:, 9:17] = c_emb.T                           # cT
    consts[:, 17:145] = np.eye(D, dtype=np.float32)     # I128 (I8 = [0:8,17:25])
    consts[0, 145:273] = 1.0                            # ones row
    consts[0:8, 273] = 0.125                            # +1/8 (mean weights)
    consts[0, 274] = fcb                                # fc bias
    consts[0:8, 275] = 1.0                              # (sigmoid-path Ln bias)
    consts[0:8, 276] = -1.0                             # negone
    consts[0:8, 277] = -0.125                           # (sigmoid-path mean)
    consts[:, 280:289] = wb.view(np.uint16).view(np.float32)  # wb bf16 packed
    return {"e_t": e_t, "consts": consts}


class _StageDoneExc(Exception):
    pass


_StageDone = _StageDoneExc()


def _build_bass(stage=2):
    import concourse.bass as bass
    import concourse.bacc as bacc
    import concourse.tile as tile
    from concourse import mybir

    AF = mybir.ActivationFunctionType
    ALU = mybir.AluOpType
    f32 = mybir.dt.float32
    bf16 = mybir.dt.bfloat16
    X = mybir.AxisListType.X
    XY = mybir.AxisListType.XY

    nc = bacc.Bacc(None, target_bir_lowering=False)
    e_t = nc.dram_tensor("e_t", [D, N], bf16, kind="ExternalInput")
    consts = nc.dram_tensor("consts", [D, CCOLS], f32, kind="ExternalInput")
    if DEBUG_OUTPUTS:
        out_d = nc.dram_tensor("out", [D, 2048], f32, kind="ExternalOutput")
    else:
        out_d = nc.dram_tensor("out", [1, 1], f32, kind="ExternalOutput")

    try:
      with tile.TileContext(nc) as tc:
        with (
            tc.tile_pool(name="sb", bufs=1) as sb,
            tc.tile_pool(name="psg", bufs=1, space="PSUM") as psg,
            tc.tile_pool(name="pss", bufs=3, space="PSUM") as pss,
        ):
            csb = sb.tile([D, CCOLS], f32, tag="csb")
            esb = [
                sb.tile([D, ETILE_J if t < 2 else N - 2 * ETILE_J], bf16, name=f"e{t}", tag=f"e{t}")
                for t in range(NTILES)
            ]

            # split the input stream across both HWDGE rings (SP + ACT)
            nc.sync.dma_start(out=csb[:, :], in_=consts[:, :])
            for t in range(NTILES):
                j0 = t * ETILE_J
                w = esb[t].shape[1]
                nsp = EDMA_SPLIT
                step = (w + nsp - 1) // nsp
                for s in range(0, w, step):
                    e = min(s + step, w)
                    eng = nc.scalar if t % 2 == 0 else nc.sync
                    eng.dma_start(
                        out=esb[t][:, s:e], in_=e_t[:, j0 + s : j0 + e]
                    )

            if stage == 0:
                dummy = sb.tile([1, 1], f32, tag="dummy")
                nc.vector.tensor_copy(dummy[:, :], esb[0][0:1, 0:1])
                nc.sync.dma_start(out=out_d[0:1, 0:1], in_=dummy[:, :])
                raise _StageDone

            wbsb = csb[:, 280:289].bitcast(mybir.dt.bfloat16)  # [D, 18] bf16 view
            I128 = csb[:, 17:145]
            I8 = csb[0:8, 17:25]
            ones128 = csb[0:1, 145:273]
            ones8 = csb[0:1, 145:153]
            one11 = csb[0:1, 145:156]
            w1col = csb[:, 0:1]
            rT = csb[:, 1:9]
            cT = csb[:, 9:17]
            fcb_ap = csb[0:1, 274:275]
            negone1 = csb[0:1, 276:277]
            negone8 = csb[0:8, 276:277]
            poseighth = csb[0:8, 273:274]
            negeighth = csb[0:8, 277:278]

            # negfcb = -fcb broadcast to all partitions (stream-time)
            psNF = pss.tile([D, 1], f32, tag="sm")
            nc.tensor.matmul(
                psNF[:, :], lhsT=ones128, rhs=fcb_ap, start=True, stop=True
            )
            negfcb_sb = sb.tile([D, 1], f32, tag="negfcb")
            nc.vector.tensor_scalar_mul(negfcb_sb[:, :], psNF[:, :], -1.0)

            # warm up the exp/ln ACT table during the DMA stream
            wt = sb.tile([1, 1], f32, tag="wt")
            wto = sb.tile([1, 1], f32, tag="wto")
            nc.vector.memset(wt[:, :], 1.0)
            nc.scalar.activation(out=wto[:, :], in_=wt[:, :], func=AF.Exp)


            # ---- per-batch bias row b17 = [rw1_b x8 | cw1_b+fcb x8 | 0] ---
            psA = pss.tile([8, 1], f32, tag="sm")
            nc.tensor.matmul(psA[:, :], lhsT=rT, rhs=w1col, start=True, stop=True)
            rw1_sb = sb.tile([8, 1], f32, tag="rw1")
            nc.vector.tensor_copy(rw1_sb[:, :], psA[:, :])

            psC = pss.tile([8, 1], f32, tag="sm")
            nc.tensor.matmul(psC[:, :], lhsT=cT, rhs=w1col, start=True, stop=False)
            nc.tensor.matmul(
                psC[:, :], lhsT=ones8, rhs=fcb_ap, start=False, stop=True
            )
            cw1_sb = sb.tile([8, 1], f32, tag="cw1")
            nc.vector.tensor_copy(cw1_sb[:, :], psC[:, :])

            psB = pss.tile([1, GC], f32, tag="sm")
            nc.vector.memset(psB[:, :], 0.0)
            nc.tensor.matmul(
                psB[0:1, 0:8], lhsT=rw1_sb[:, :], rhs=I8, start=True, stop=True
            )
            nc.tensor.matmul(
                psB[0:1, 8:16], lhsT=cw1_sb[:, :], rhs=I8, start=True, stop=True
            )
            b17_sb = sb.tile([1, GC], f32, tag="b17")
            nc.vector.tensor_copy(b17_sb[:, :], psB[:, :])

            # replicate to [1, 187] so one k=1 matmul biases a whole tile
            b187_sb = sb.tile([1, TCOLS], f32, tag="b187")
            for q in range(NQ):
                nc.vector.tensor_copy(
                    b187_sb[0:1, q * GC : (q + 1) * GC], b17_sb[:, :]
                )

            # ---- grid: bias first (start=True sets every element +
            # has_written for the whole tile), then the e-chunk matmuls
            # accumulate on top with start=False.  start=True clears
            # has_written bank-wide, so the bias write must be the only
            # start=True targeting each grid bank.
            ps = [psg.tile([D, TCOLS], f32, name=f"g{t}", tag=f"g{t}") for t in range(NTILES)]
            for t in range(NTILES):
                nc.tensor.matmul(
                    ps[t][:, :],
                    lhsT=ones128,
                    rhs=b187_sb[:, :],
                    start=True,
                    stop=False,
                    skip_group_check=True,
                )
                for q in range(NQ):
                    k = t * NQ + q
                    m = 4 if k == NCHUNKS - 1 else 128
                    off = 128 * k - t * ETILE_J
                    nc.tensor.matmul(
                        ps[t][0:m, q * GC : (q + 1) * GC],
                        lhsT=esb[t][:, off : off + m],
                        rhs=wbsb[:, 0:GC],
                        start=False,
                        stop=True,
                        skip_group_check=True,
                    )

            # ---- ci_max = max_j s2[j]  (raw col 16 of each chunk block) ---
            ci3 = sb.tile([D, NTILES], f32, tag="ci3")
            for t in range(NTILES):
                v = ps[t][:, :].rearrange("p (q c) -> p q c", c=GC)[:, :, 16:17]
                nc.vector.tensor_reduce(
                    ci3[:, t : t + 1], v, axis=XY, op=ALU.max
                )
            ci_sb = sb.tile([D, 1], f32, tag="ci")
            nc.vector.tensor_reduce(ci_sb[:, :], ci3[:, :], axis=X, op=ALU.max)
            psT1 = pss.tile([1, D], f32, tag="sm")
            nc.tensor.transpose(psT1[:, :], ci_sb[:, :], I128)
            m0_sb = sb.tile([1, 1], f32, tag="m0")
            nc.vector.tensor_reduce(m0_sb[:, :], psT1[:, :], axis=X, op=ALU.max)
            if stage == 1:
                nc.sync.dma_start(out=out_d[0:1, 0:1], in_=m0_sb[:, :])

            if stage < 2:
                raise _StageDone
            # ---- exp of the whole grid (no cimax dependency -> runs
            # during the DMA stream), v-side +1 also stream-time ----------
            gE = sb.tile([D, NTILES * NQ * 16], f32, tag="gE")
            q2b = sb.tile([D, NTILES * NQ * 8], f32, tag="q2b")
            for t in range(NTILES):
                pv = ps[t][:, :].rearrange("p (q c) -> p q c", c=GC)
                oE = gE[:, t * 176 : (t + 1) * 176].rearrange(
                    "p (q c) -> p q c", c=16
                )
                nc.scalar.activation(
                    out=oE, in_=pv[:, :, 0:16], func=AF.Exp, bias=0.0, scale=-1.0
                )
                ev = gE[:, t * 176 : (t + 1) * 176].rearrange(
                    "p (q c) -> p q c", c=16
                )[:, :, 8:16]
                nc.vector.tensor_scalar_add(
                    q2b[:, t * 88 : (t + 1) * 88].rearrange(
                        "p (q c) -> p q c", c=8
                    ),
                    ev,
                    1.0,
                )

            # ---- ef = exp(-(fcb + ci_max)) broadcast over partitions ------
            psCI = pss.tile([D, 1], f32, tag="sm")
            nc.tensor.matmul(
                psCI[:, :], lhsT=ones128, rhs=m0_sb[:, :], start=True, stop=True
            )
            ef_sb = sb.tile([D, 1], f32, tag="ef")
            nc.scalar.activation(
                out=ef_sb[:, :], in_=psCI[:, :], func=AF.Exp,
                bias=negfcb_sb[:, :], scale=-1.0,
            )
            if stage == 3:
                nc.sync.dma_start(out=out_d[0:1, 0:1], in_=ef_sb[0:1, :])
                raise _StageDone

            # ---- q1 = E_u*ef + 1 (fused), P = q1*q2, min over chunks ------
            # gE/q1b/q2b are contiguous across the 3 psum tiles, so the
            # cimax-dependent tail is just 3 whole-grid DVE ops.
            q1b = sb.tile([D, NTILES * NQ * 8], f32, tag="q1b")
            nc.vector.tensor_scalar(
                q1b[:, :].rearrange("p (tq c) -> p tq c", c=8),
                gE[:, :].rearrange("p (tq c) -> p tq c", c=16)[:, :, 0:8],
                ef_sb[:, :],
                1.0,
                op0=ALU.mult,
                op1=ALU.add,
            )
            nc.vector.tensor_mul(q1b[:, :], q1b[:, :], q2b[:, :])
            mnp_sb = sb.tile([D, 8], f32, tag="mnp")
            nc.vector.tensor_reduce(
                mnp_sb[:, :],
                q1b[:, :].rearrange("p (tq b) -> p b tq", b=8),
                axis=X,
                op=ALU.min,
            )
            if stage == 4:
                nc.sync.dma_start(out=out_d[0:1, 0:1], in_=mnp_sb[0:1, 0:1])
                raise _StageDone

            # ---- global min over partitions, then the loss ---------------
            psT2 = pss.tile([8, D], f32, tag="sm")
            nc.tensor.transpose(psT2[:, :], mnp_sb[:, :], I128)
            mn_sb = sb.tile([8, 1], f32, tag="mnb")
            nc.vector.tensor_reduce(mn_sb[:, :], psT2[:, :], axis=X, op=ALU.min)
            if stage == 5:
                nc.sync.dma_start(out=out_d[0:1, 0:1], in_=mn_sb[0:1, :])
                raise _StageDone

            if stage == 55:
                nc.sync.dma_start(out=out_d[0:1, 0:1], in_=mn_sb[0:1, :])
                raise _StageDone
            t1_sb = sb.tile([8, 1], f32, tag="t1")
            t2_sb = sb.tile([8, 1], f32, tag="t2")
            nc.scalar.activation(out=t1_sb[:, :], in_=mn_sb[:, :], func=AF.Ln)
            nc.scalar.activation(
                out=t2_sb[:, :], in_=mn_sb[:, :], func=AF.Ln, bias=negone8
            )
            if stage == 6:
                nc.sync.dma_start(out=out_d[0:1, 0:1], in_=t1_sb[0:1, :])
                raise _StageDone

            if DEBUG_OUTPUTS:
                psdump = sb.tile([D, NTILES * TCOLS], f32, tag="psdump")
                for t in range(NTILES):
                    nc.vector.tensor_copy(
                        psdump[:, t * TCOLS : (t + 1) * TCOLS], ps[t][:, :]
                    )
                nc.sync.dma_start(out=out_d[:, 561:1122], in_=psdump[:, :])
                nc.sync.dma_start(out=out_d[0:1, 1914:1931], in_=b17_sb[:, :])
                nc.sync.dma_start(out=out_d[0:1, 1931:1932], in_=m0_sb[:, :])
                nc.sync.dma_start(out=out_d[0:8, 1940:1941], in_=mn_sb[:, :])

            # loss = (1/8)*sum_b t1 - (1/8)*sum_b t2, folded into one
            # accumulating matmul pair (no separate DVE subtract).
            psL = pss.tile([1, 1], f32, tag="sm")
            nc.tensor.matmul(
                psL[:, :], lhsT=t1_sb[:, :], rhs=poseighth, start=True, stop=False
            )
            nc.tensor.matmul(
                psL[:, :], lhsT=t2_sb[:, :], rhs=negeighth, start=False, stop=True
            )
            out_sb = sb.tile([1, 1], f32, tag="osb")
            nc.scalar.copy(out_sb[:, :], psL[:, :])
            if DEBUG_OUTPUTS:
                nc.sync.dma_start(out=out_d[0:1, 1960:1961], in_=out_sb[:, :])
            else:
                nc.sync.dma_start(out=out_d[:, :], in_=out_sb[:, :])

    except _StageDoneExc:
        pass
    nc.compile()
    # Both Exp and Ln live in act table set 6 (natural_log_exp_and_others);
    # the table-load pass picks set 0 for Exp then swaps to set 5 for Ln,
    # costing a second ~1.3us table load in the tail.  Use set 6 once.
    first = True
    for b in nc.main_func.blocks:
        drop = []
        for i in b.instructions:
            if isinstance(i, mybir.InstLoadActFuncSet):
                if first:
                    i.act_func_set_id = 6
                    first = False
                else:
                    si = i.sync_info
                    assert si is None or (not si.on_wait and not si.on_update)
                    drop.append(i)
        for i in drop:
            b.instructions.remove(i)
    return nc


def kernel(**inputs) -> np.ndarray:
    global LAST_EXEC_NS
    from concourse.bass_utils import run_bass_kernel_spmd

    dev_inputs = _build_device_inputs(**inputs)
    if "nc" not in _CACHE:
        _CACHE["nc"] = _build_bass()
    nc = _CACHE["nc"]

    n_cores = 8
    in_maps = [dict(dev_inputs) for _ in range(n_cores)]
    res = run_bass_kernel_spmd(nc, in_maps, core_ids=list(range(n_cores)))
    LAST_EXEC_NS = res.exec_time_ns
    _CACHE["last_results"] = res.results
    if DEBUG_OUTPUTS:
        return np.asarray(res.results[0]["out"], np.float32)
    out = np.asarray(res.results[0]["out"], np.float32).reshape(())
    return out
